# revision 9
# baseline (speedup 1.0000x reference)
"""AFResampler Trainium2 kernel.

Math: the reference's _normalize() is shift-invariant, so all 9 (oh, ow)
offsets produce the SAME sampling grid; the MLP-weighted sum then cancels
exactly (value / w_sum == single grid_sample).  With H=W=256 -> 128, the
grid sample reduces to a separable 2x bilinear downsample:

    r[i]    = (1 - i/127) * feat[2i]   + (i/127) * feat[2i+1]      (rows)
    r[:, j] = (1 - j/127) * rr[:, 2j]  + (j/127) * rr[:, 2j+1]     (cols)

followed by out = conv3x3(conv3x3(r, w1)+b1, w2)+b2.  Bias contributions
are feat-independent and added on the host.

Device layout: one batch element per NeuronCore (8-way data parallel).
On-chip, partitions = (row-parity, channel): p<64 holds channel c's EVEN
r-rows, p>=64 the ODD r-rows (O-array slot s = r[2s-1], so conv taps for
one output row always read a single slot index across both halves).
3x3 convs run as 2 matmuls per kernel-column (one K=128, one K=64)
per 4-row output tile, accumulated in PSUM, bf16 operands / f32 psum.
"""

import numpy as np

import concourse.bass as bass
import concourse.bacc as bacc
import concourse.mybir as mybir
from concourse.tile import TileContext
from concourse.bass_utils import run_bass_kernel_spmd

BF16 = mybir.dt.bfloat16
F32 = mybir.dt.float32
NP_BF16 = np.dtype(mybir.dt.np(BF16))

C = 64          # channels
HO = 128        # output spatial
NSLOT = 65      # parity slots incl pad
XPAD = 130      # 128 + 2 zero cols
SCHUNK = 8      # slots computed per resample chunk
NCHUNK = 64 // SCHUNK

# wconst free-dim offsets (one [128, F] tensor holds all weights)
O_W1A = 0            # [128, 64]  f32-as-bf16? kept bf16; stage1 A weights
O_W1B = 64           # [128, 64]
O_WINT = 128         # [128, 256] interleaved stage2 weights
O_C1 = 384           # 12 slabs x 128
O_C2 = O_C1 + 12 * 128   # 12 slabs x 4
WF = O_C2 + 12 * 4       # total free size


def _build_wconst(conv1_w, conv2_w):
    """Assemble the [128, WF] bf16 constant tensor."""
    wc = np.zeros((128, WF), np.float32)
    s = np.arange(64, dtype=np.float32)
    par = (np.arange(128) // 64).astype(np.float32)[:, None]  # [128,1]
    yw = (2.0 * s[None, :] + par) / 127.0                     # B weight per (p, slot)
    wc[:, O_W1A:O_W1A + 64] = 1.0 - yw
    wc[:, O_W1B:O_W1B + 64] = yw
    j = np.arange(128, dtype=np.float32) / 127.0
    wint = np.zeros(256, np.float32)
    wint[0::2] = 1.0 - j
    wint[1::2] = j
    wc[:, O_WINT:O_WINT + 256] = wint[None, :]

    def t(w, kh, kw):  # lhsT block [cin, cout]
        return w[:, :, kh, kw].T.astype(np.float32)

    # conv1 slabs (12 x [128, 128]).  M-cols 0..63 for even tiles,
    # 64..127 for odd tiles (psum partition offset trick).
    for dw in range(3):
        ea = np.zeros((128, 128), np.float32)
        ea[0:64, 0:64] = t(conv1_w, 1, dw)     # E half: tap dh=0
        ea[64:128, 0:64] = t(conv1_w, 0, dw)   # O half: tap dh=-1
        wc[:, O_C1 + dw * 128: O_C1 + dw * 128 + 128] = ea
        eb = np.zeros((128, 128), np.float32)
        eb[64:128, 0:64] = t(conv1_w, 2, dw)   # O slot s+1: tap dh=+1
        wc[:, O_C1 + (3 + dw) * 128: O_C1 + (4 + dw) * 128] = eb
        oa = np.zeros((128, 128), np.float32)
        oa[0:64, 64:128] = t(conv1_w, 2, dw)   # E slot s+1: tap dh=+1
        oa[64:128, 64:128] = t(conv1_w, 1, dw) # O slot s+1: tap dh=0
        wc[:, O_C1 + (6 + dw) * 128: O_C1 + (7 + dw) * 128] = oa
        ob = np.zeros((128, 128), np.float32)
        ob[0:64, 64:128] = t(conv1_w, 0, dw)   # E slot s: tap dh=-1
        wc[:, O_C1 + (9 + dw) * 128: O_C1 + (10 + dw) * 128] = ob

    # conv2 slabs (12 x [128, 4]), M = 3, no offset trick needed
    for dw in range(3):
        ea = np.zeros((128, 4), np.float32)
        ea[0:64, 0:3] = t(conv2_w, 1, dw)
        ea[64:128, 0:3] = t(conv2_w, 0, dw)
        wc[:, O_C2 + dw * 4: O_C2 + dw * 4 + 4] = ea
        eb = np.zeros((128, 4), np.float32)
        eb[64:128, 0:3] = t(conv2_w, 2, dw)
        wc[:, O_C2 + (3 + dw) * 4: O_C2 + (4 + dw) * 4] = eb
        oa = np.zeros((128, 4), np.float32)
        oa[0:64, 0:3] = t(conv2_w, 2, dw)
        oa[64:128, 0:3] = t(conv2_w, 1, dw)
        wc[:, O_C2 + (6 + dw) * 4: O_C2 + (7 + dw) * 4] = oa
        ob = np.zeros((128, 4), np.float32)
        ob[0:64, 0:3] = t(conv2_w, 0, dw)
        wc[:, O_C2 + (9 + dw) * 4: O_C2 + (10 + dw) * 4] = ob
    return wc.astype(NP_BF16)


def _emit_conv1_tile(nc, wc, r_par, h_par, psum_pool, t, parity):
    """One 4-output-row conv1 tile (rows 2s [+1], s = 4t..4t+3)."""
    if parity == 0:  # even rows -> h_par E half (partitions 0..63)
        ps = psum_pool.tile([64, 4, 128], F32, tag="p1e", bufs=2)
        for dw in range(3):
            lhA = wc[:, bass.ds(O_C1 + dw * 128, 64)]
            rhA = r_par[:, bass.ds(4 * t, 4), bass.ds(dw, 128)]
            nc.tensor.matmul(ps[:], lhA, rhA, start=(dw == 0), stop=False)
            lhB = wc[64:128, bass.ds(O_C1 + (3 + dw) * 128, 64)]
            rhB = r_par[64:128, bass.ds(4 * t + 1, 4), bass.ds(dw, 128)]
            nc.tensor.matmul(ps[:], lhB, rhB, start=False, stop=(dw == 2))
        nc.scalar.activation(
            h_par[0:64, bass.ds(4 * t, 4), 1:129], ps[:],
            mybir.ActivationFunctionType.Copy)
    else:  # odd rows -> h_par O half (partitions 64..127), slots 4t+1..4t+4
        ps = psum_pool.tile([128, 4, 128], F32, tag="p1o", bufs=2)
        for dw in range(3):
            lhA = wc[:, bass.ds(O_C1 + (6 + dw) * 128, 128)]
            rhA = r_par[:, bass.ds(4 * t + 1, 4), bass.ds(dw, 128)]
            nc.tensor.matmul(ps[:], lhA, rhA, start=(dw == 0), stop=False)
            lhB = wc[0:64, bass.ds(O_C1 + (9 + dw) * 128, 128)]
            rhB = r_par[0:64, bass.ds(4 * t, 4), bass.ds(dw, 128)]
            nc.tensor.matmul(ps[:], lhB, rhB, start=False, stop=(dw == 2))
        nc.scalar.activation(
            h_par[64:128, bass.ds(4 * t + 1, 4), 1:129], ps[64:128, :, :],
            mybir.ActivationFunctionType.Copy)


def _emit_conv2_tile(nc, wc, h_par, out_view, out_pool, psum_pool, t, parity):
    ps = psum_pool.tile([3, 4, 128], F32, tag="p2", bufs=2)
    if parity == 0:
        for dw in range(3):
            lhA = wc[:, bass.ds(O_C2 + dw * 4, 3)]
            rhA = h_par[:, bass.ds(4 * t, 4), bass.ds(dw, 128)]
            nc.tensor.matmul(ps[:], lhA, rhA, start=(dw == 0), stop=False)
            lhB = wc[64:128, bass.ds(O_C2 + (3 + dw) * 4, 3)]
            rhB = h_par[64:128, bass.ds(4 * t + 1, 4), bass.ds(dw, 128)]
            nc.tensor.matmul(ps[:], lhB, rhB, start=False, stop=(dw == 2))
    else:
        for dw in range(3):
            lhA = wc[:, bass.ds(O_C2 + (6 + dw) * 4, 3)]
            rhA = h_par[:, bass.ds(4 * t + 1, 4), bass.ds(dw, 128)]
            nc.tensor.matmul(ps[:], lhA, rhA, start=(dw == 0), stop=False)
            lhB = wc[0:64, bass.ds(O_C2 + (9 + dw) * 4, 3)]
            rhB = h_par[0:64, bass.ds(4 * t, 4), bass.ds(dw, 128)]
            nc.tensor.matmul(ps[:], lhB, rhB, start=False, stop=(dw == 2))
    ot = out_pool.tile([3, 4, 128], F32, tag="ot", bufs=3)
    nc.scalar.activation(ot[:], ps[:], mybir.ActivationFunctionType.Copy)
    nc.sync.dma_start(out=out_view[:, bass.ds(4 * t, 4), parity, :], in_=ot[:])


def build_program():
    nc = bacc.Bacc(trn_type="TRN2")
    feat = nc.dram_tensor("feat", [C, 256, 256], F32, kind="ExternalInput")
    wconst = nc.dram_tensor("wconst", [128, WF], BF16, kind="ExternalInput")
    out = nc.dram_tensor("out", [3, HO, HO], F32, kind="ExternalOutput")
    # feat viewed as [q, c, s4, x] with y = 4*s4 + q
    feat_v = feat[:].rearrange("c (s q) x -> q c s x", q=4)
    out_view = out[:].rearrange("co (s two) x -> co s two x", two=2)

    with TileContext(nc) as tc:
        with (
            tc.tile_pool(name="const", bufs=1) as cpool,
            tc.tile_pool(name="persist", bufs=1) as ppool,
            tc.tile_pool(name="ld", bufs=2) as ldpool,
            tc.tile_pool(name="st1", bufs=2) as stpool,
            tc.tile_pool(name="outp", bufs=3) as out_pool,
            tc.tile_pool(name="psum", bufs=2, space="PSUM") as psum_pool,
        ):
            wc = cpool.tile([128, WF], BF16)
            nc.sync.dma_start(out=wc[:], in_=wconst[:])
            # Engine-local copies of the resample weights: keeps the
            # wconst-DMA semaphore off the stage TT ops (gen3 TT has only
            # 2 sync-wait slots, and each load tile already needs 2).
            wc_v = cpool.tile([128, O_C1], BF16)
            wc_g = cpool.tile([128, 64], BF16)
            nc.vector.tensor_copy(out=wc_v[:], in_=wc[:, 0:O_C1])
            nc.gpsimd.tensor_copy(out=wc_g[:], in_=wc[:, O_W1B:O_W1B + 64])

            r_par = ppool.tile([128, NSLOT, XPAD], BF16)
            h_par = ppool.tile([128, NSLOT, XPAD], BF16)
            for tile in (r_par, h_par):
                nc.vector.memset(tile[0:64, 64, :], 0.0)    # E slot 64 pad
                nc.vector.memset(tile[64:128, 0, :], 0.0)   # O slot 0 pad
                nc.vector.memset(tile[:, :, 0], 0.0)        # left col pad
                nc.vector.memset(tile[:, :, 129], 0.0)      # right col pad

            e_done = o_done = c2e = c2o = 0

            def conv_progress(e_max, o_max):
                nonlocal e_done, o_done, c2e, c2o
                while e_done < e_max:
                    _emit_conv1_tile(nc, wc, r_par, h_par, psum_pool, e_done, 0)
                    e_done += 1
                while o_done < o_max:
                    _emit_conv1_tile(nc, wc, r_par, h_par, psum_pool, o_done, 1)
                    o_done += 1
                while c2e < min(e_done, o_done):
                    _emit_conv2_tile(nc, wc, h_par, out_view, out_pool,
                                     psum_pool, c2e, 0)
                    c2e += 1
                while c2o < min(e_done - 1, o_done):
                    _emit_conv2_tile(nc, wc, h_par, out_view, out_pool,
                                     psum_pool, c2o, 1)
                    c2o += 1

            mul = mybir.AluOpType.mult
            add = mybir.AluOpType.add
            for kc in range(NCHUNK):
                s0 = SCHUNK * kc
                fa = ldpool.tile([128, SCHUNK, 256], F32, tag="fa", bufs=2)
                fb = ldpool.tile([128, SCHUNK, 256], F32, tag="fb", bufs=2)
                # partition p = (par, c); row = 4s + 2par (+1 for fb)
                nc.sync.dma_start(out=fa[0:64], in_=feat_v[0, :, bass.ds(s0, SCHUNK), :])
                nc.sync.dma_start(out=fa[64:128], in_=feat_v[2, :, bass.ds(s0, SCHUNK), :])
                nc.sync.dma_start(out=fb[0:64], in_=feat_v[1, :, bass.ds(s0, SCHUNK), :])
                nc.sync.dma_start(out=fb[64:128], in_=feat_v[3, :, bass.ds(s0, SCHUNK), :])

                wa = wc_v[:, bass.ds(O_W1A + s0, SCHUNK)].unsqueeze(2).broadcast_to(
                    [128, SCHUNK, 256])
                wb = wc_g[:, bass.ds(s0, SCHUNK)].unsqueeze(2).broadcast_to(
                    [128, SCHUNK, 256])
                wi = wc_v[:, bass.ds(O_WINT, 256)].unsqueeze(1).broadcast_to(
                    [128, SCHUNK, 256])

                t1 = stpool.tile([128, SCHUNK, 256], BF16, tag="t1", bufs=2)
                t2 = stpool.tile([128, SCHUNK, 256], BF16, tag="t2", bufs=2)
                t3 = stpool.tile([128, SCHUNK, 256], BF16, tag="t3", bufs=2)
                tp = stpool.tile([128, SCHUNK, 256], BF16, tag="tp", bufs=2)
                nc.vector.tensor_tensor(out=t1[:], in0=fa[:], in1=wa, op=mul)
                nc.gpsimd.tensor_tensor(out=t2[:], in0=fb[:], in1=wb, op=mul)
                nc.vector.tensor_tensor(out=t3[:], in0=t1[:], in1=t2[:], op=add)
                nc.vector.tensor_tensor(out=tp[:], in0=t3[:], in1=wi, op=mul)
                # pairwise sum of adjacent cols -> r_par (E half / O half +1)
                tpe = tp[:, :, 0::2]
                tpo = tp[:, :, 1::2]
                nc.vector.tensor_tensor(
                    out=r_par[0:64, bass.ds(s0, SCHUNK), 1:129],
                    in0=tpe[0:64], in1=tpo[0:64], op=add)
                nc.vector.tensor_tensor(
                    out=r_par[64:128, bass.ds(s0 + 1, SCHUNK), 1:129],
                    in0=tpe[64:128], in1=tpo[64:128], op=add)

                if kc < NCHUNK - 1:
                    conv_progress(min(2 * kc + 2, 16), min(2 * kc + 1, 16))
            conv_progress(16, 16)
            # final odd conv2 tile: its E-slot-64 halo is the memset pad,
            # not a 17th conv1 tile, so emit it explicitly
            while c2o < 16:
                _emit_conv2_tile(nc, wc, h_par, out_view, out_pool,
                                 psum_pool, c2o, 1)
                c2o += 1

    nc.finalize()
    return nc


_PROG = None


def _get_program():
    global _PROG
    if _PROG is None:
        _PROG = build_program()
    return _PROG


def _bias_map(conv1_b, conv2_b, conv2_w):
    """Feat-independent bias contribution of both convs, [3,128,128]."""
    if not conv1_b.any() and not conv2_b.any():
        return None
    h = np.broadcast_to(conv1_b[:, None, None], (C, HO, HO)).astype(np.float32)
    hp = np.zeros((C, HO + 2, HO + 2), np.float32)
    hp[:, 1:-1, 1:-1] = h
    o = np.zeros((3, HO, HO), np.float32)
    for kh in range(3):
        for kw in range(3):
            o += np.einsum("oc,chw->ohw", conv2_w[:, :, kh, kw],
                           hp[:, kh:kh + HO, kw:kw + HO])
    return o + conv2_b[:, None, None]


def kernel(**inputs):
    feat = np.ascontiguousarray(np.asarray(inputs["feat"], dtype=np.float32))
    conv1_w = np.asarray(inputs["conv1_w"], np.float32)
    conv1_b = np.asarray(inputs["conv1_b"], np.float32)
    conv2_w = np.asarray(inputs["conv2_w"], np.float32)
    conv2_b = np.asarray(inputs["conv2_b"], np.float32)

    wc = _build_wconst(conv1_w, conv2_w)
    nc = _get_program()
    in_maps = [{"feat": feat[b], "wconst": wc} for b in range(feat.shape[0])]
    import os
    trace = bool(int(os.environ.get("AFR_TRACE", "0")))
    res = run_bass_kernel_spmd(nc, in_maps, core_ids=list(range(8)),
                               trace=trace)
    if trace:
        print(f"HW exec time: {res.exec_time_ns} ns")
    outs = np.stack([m["out"].reshape(3, HO, HO) for m in res.results])
    bm = _bias_map(conv1_b, conv2_b, conv2_w)
    if bm is not None:
        outs = outs + bm[None]
    return outs.astype(np.float32)


if __name__ == "__main__":
    prog = build_program()
    print("program built OK")


# revision 14
# speedup vs baseline: 1.6398x; 1.6398x over previous
"""AFResampler Trainium2 kernel.

Math: the reference's _normalize() is shift-invariant, so all 9 (oh, ow)
offsets produce the SAME sampling grid; the MLP-weighted sum then cancels
exactly (value / w_sum == single grid_sample).  With H=W=256 -> 128, the
grid sample reduces to a separable 2x bilinear downsample:

    r[i]    = (1 - i/127) * feat[2i]   + (i/127) * feat[2i+1]      (rows)
    r[:, j] = (1 - j/127) * rr[:, 2j]  + (j/127) * rr[:, 2j+1]     (cols)

followed by out = conv3x3(conv3x3(r, w1)+b1, w2)+b2.  Bias contributions
are feat-independent and added on the host.

Device layout: one batch element per NeuronCore (8-way data parallel).
On-chip, partitions = (row-parity, channel): p<64 holds channel c's EVEN
r-rows, p>=64 the ODD r-rows (O-array slot s = r[2s-1], so conv taps for
one output row always read a single slot index across both halves).
3x3 convs run as 2 matmuls per kernel-column (one K=128, one K=64)
per 4-row output tile, accumulated in PSUM, bf16 operands / f32 psum.
"""

import numpy as np

import concourse.bass as bass
import concourse.bacc as bacc
import concourse.mybir as mybir
from concourse.tile import TileContext
from concourse.bass_utils import run_bass_kernel_spmd

BF16 = mybir.dt.bfloat16
F32 = mybir.dt.float32
NP_BF16 = np.dtype(mybir.dt.np(BF16))

C = 64          # channels
HO = 128        # output spatial
NSLOT = 65      # parity slots incl pad
XPAD = 130      # 128 + 2 zero cols
SCHUNK = 8      # slots computed per resample chunk
NCHUNK = 64 // SCHUNK

# wconst free-dim offsets (one [128, F] tensor holds all weights)
O_W1A = 0            # [128, 64]  stage1 A weights
O_W1B = 64           # [128, 64]
O_WINT = 128         # [128, 256] interleaved stage2 weights
O_C1 = 384           # 12 slabs x 128
C2W = 68             # conv2 slab width (M cols 0..2 even / 64..66 odd)
O_C2 = O_C1 + 12 * 128   # 12 slabs x C2W
WF = O_C2 + 12 * C2W     # total free size


def _build_wconst(conv1_w, conv2_w):
    """Assemble the [128, WF] bf16 constant tensor."""
    wc = np.zeros((128, WF), np.float32)
    s = np.arange(64, dtype=np.float32)
    par = (np.arange(128) // 64).astype(np.float32)[:, None]  # [128,1]
    yw = (2.0 * s[None, :] + par) / 127.0                     # B weight per (p, slot)
    wc[:, O_W1A:O_W1A + 64] = 1.0 - yw
    wc[:, O_W1B:O_W1B + 64] = yw
    j = np.arange(128, dtype=np.float32) / 127.0
    wint = np.zeros(256, np.float32)
    wint[0::2] = 1.0 - j
    wint[1::2] = j
    wc[:, O_WINT:O_WINT + 256] = wint[None, :]

    def t(w, kh, kw):  # lhsT block [cin, cout]
        return w[:, :, kh, kw].T.astype(np.float32)

    # conv1 slabs (12 x [128, 128]).  M-cols 0..63 for even tiles,
    # 64..127 for odd tiles (psum partition offset trick).
    for dw in range(3):
        ea = np.zeros((128, 128), np.float32)
        ea[0:64, 0:64] = t(conv1_w, 1, dw)     # E half: tap dh=0
        ea[64:128, 0:64] = t(conv1_w, 0, dw)   # O half: tap dh=-1
        wc[:, O_C1 + dw * 128: O_C1 + dw * 128 + 128] = ea
        eb = np.zeros((128, 128), np.float32)
        eb[64:128, 0:64] = t(conv1_w, 2, dw)   # O slot s+1: tap dh=+1
        wc[:, O_C1 + (3 + dw) * 128: O_C1 + (4 + dw) * 128] = eb
        oa = np.zeros((128, 128), np.float32)
        oa[0:64, 64:128] = t(conv1_w, 2, dw)   # E slot s+1: tap dh=+1
        oa[64:128, 64:128] = t(conv1_w, 1, dw) # O slot s+1: tap dh=0
        wc[:, O_C1 + (6 + dw) * 128: O_C1 + (7 + dw) * 128] = oa
        ob = np.zeros((128, 128), np.float32)
        ob[0:64, 64:128] = t(conv1_w, 0, dw)   # E slot s: tap dh=-1
        wc[:, O_C1 + (9 + dw) * 128: O_C1 + (10 + dw) * 128] = ob

    # conv2 slabs (12 x [128, 68]).  Even tiles use M-cols 0..2, odd tiles
    # 64..66 so even/odd matmuls col-pack into disjoint array col-groups.
    for dw in range(3):
        ea = np.zeros((128, C2W), np.float32)
        ea[0:64, 0:3] = t(conv2_w, 1, dw)
        ea[64:128, 0:3] = t(conv2_w, 0, dw)
        wc[:, O_C2 + dw * C2W: O_C2 + (dw + 1) * C2W] = ea
        eb = np.zeros((128, C2W), np.float32)
        eb[64:128, 0:3] = t(conv2_w, 2, dw)
        wc[:, O_C2 + (3 + dw) * C2W: O_C2 + (4 + dw) * C2W] = eb
        oa = np.zeros((128, C2W), np.float32)
        oa[0:64, 64:67] = t(conv2_w, 2, dw)
        oa[64:128, 64:67] = t(conv2_w, 1, dw)
        wc[:, O_C2 + (6 + dw) * C2W: O_C2 + (7 + dw) * C2W] = oa
        ob = np.zeros((128, C2W), np.float32)
        ob[0:64, 64:67] = t(conv2_w, 0, dw)
        wc[:, O_C2 + (9 + dw) * C2W: O_C2 + (10 + dw) * C2W] = ob
    return wc.astype(NP_BF16)


def _emit_conv1_pair(nc, wc, r_par, h_par, psum_pool, t):
    """Even+odd conv1 tiles for slot-group t, col-packed into one psum bank
    (even rows -> psum partitions 0..63 / array cols 0..63; odd rows ->
    64..127).  Matmuls of the two halves interleave so the PE runs them
    concurrently in disjoint column groups."""
    ps = psum_pool.tile([128, 4, 128], F32, tag="p1", bufs=3,
                        name=f"ps1_{t}")
    for dw in range(3):
        # even tile: taps dh=0 (E slot s) + dh=-1 (O slot s), K=128
        nc.tensor.matmul(ps[0:64],
                         wc[:, bass.ds(O_C1 + dw * 128, 64)],
                         r_par[:, bass.ds(4 * t, 4), bass.ds(dw, 128)],
                         start=(dw == 0), stop=False)
        # odd tile: taps dh=+1 (E slot s+1) + dh=0 (O slot s+1), K=128
        nc.tensor.matmul(ps[64:128],
                         wc[:, bass.ds(O_C1 + (6 + dw) * 128 + 64, 64)],
                         r_par[:, bass.ds(4 * t + 1, 4), bass.ds(dw, 128)],
                         start=(dw == 0), stop=False)
        # even tile: tap dh=+1 (O slot s+1), K=64
        nc.tensor.matmul(ps[0:64],
                         wc[64:128, bass.ds(O_C1 + (3 + dw) * 128, 64)],
                         r_par[64:128, bass.ds(4 * t + 1, 4), bass.ds(dw, 128)],
                         start=False, stop=(dw == 2))
        # odd tile: tap dh=-1 (E slot s), K=64
        nc.tensor.matmul(ps[64:128],
                         wc[0:64, bass.ds(O_C1 + (9 + dw) * 128 + 64, 64)],
                         r_par[0:64, bass.ds(4 * t, 4), bass.ds(dw, 128)],
                         start=False, stop=(dw == 2))
    nc.scalar.activation(
        h_par[0:64, bass.ds(4 * t, 4), 1:129], ps[0:64, :, :],
        mybir.ActivationFunctionType.Copy)
    nc.vector.tensor_copy(
        out=h_par[64:128, bass.ds(4 * t + 1, 4), 1:129], in_=ps[64:128, :, :])


def _emit_conv2_pair(nc, wc, h_par, out_view, out_pool, psum_pool, t):
    ps = psum_pool.tile([128, 4, 128], F32, tag="p2", bufs=3,
                        name=f"ps2_{t}")
    for dw in range(3):
        nc.tensor.matmul(ps[0:3],
                         wc[:, bass.ds(O_C2 + dw * C2W, 3)],
                         h_par[:, bass.ds(4 * t, 4), bass.ds(dw, 128)],
                         start=(dw == 0), stop=False)
        nc.tensor.matmul(ps[64:67],
                         wc[:, bass.ds(O_C2 + (6 + dw) * C2W + 64, 3)],
                         h_par[:, bass.ds(4 * t + 1, 4), bass.ds(dw, 128)],
                         start=(dw == 0), stop=False)
        nc.tensor.matmul(ps[0:3],
                         wc[64:128, bass.ds(O_C2 + (3 + dw) * C2W, 3)],
                         h_par[64:128, bass.ds(4 * t + 1, 4), bass.ds(dw, 128)],
                         start=False, stop=(dw == 2))
        nc.tensor.matmul(ps[64:67],
                         wc[0:64, bass.ds(O_C2 + (9 + dw) * C2W + 64, 3)],
                         h_par[0:64, bass.ds(4 * t, 4), bass.ds(dw, 128)],
                         start=False, stop=(dw == 2))
    ot = out_pool.tile([67, 4, 128], F32, tag="ot", bufs=3, name=f"ot_{t}")
    nc.scalar.activation(ot[0:3], ps[0:3, :, :],
                         mybir.ActivationFunctionType.Copy)
    nc.scalar.activation(ot[64:67], ps[64:67, :, :],
                         mybir.ActivationFunctionType.Copy)
    nc.sync.dma_start(out=out_view[:, bass.ds(4 * t, 4), 0, :], in_=ot[0:3])
    nc.sync.dma_start(out=out_view[:, bass.ds(4 * t, 4), 1, :], in_=ot[64:67])


def build_program():
    nc = bacc.Bacc(trn_type="TRN2")
    feat = nc.dram_tensor("feat", [C, 256, 256], F32, kind="ExternalInput")
    wconst = nc.dram_tensor("wconst", [128, WF], BF16, kind="ExternalInput")
    out = nc.dram_tensor("out", [3, HO, HO], F32, kind="ExternalOutput")
    # feat viewed as [q, c, s4, x] with y = 4*s4 + q
    feat_v = feat[:].rearrange("c (s q) x -> q c s x", q=4)
    out_view = out[:].rearrange("co (s two) x -> co s two x", two=2)

    with TileContext(nc) as tc:
        with (
            tc.tile_pool(name="const", bufs=1) as cpool,
            tc.tile_pool(name="persist", bufs=1) as ppool,
            tc.tile_pool(name="ld", bufs=2) as ldpool,
            tc.tile_pool(name="st1", bufs=2) as stpool,
            tc.tile_pool(name="outp", bufs=3) as out_pool,
            tc.tile_pool(name="psum", bufs=2, space="PSUM") as psum_pool,
        ):
            wc = cpool.tile([128, WF], BF16)
            nc.sync.dma_start(out=wc[:], in_=wconst[:])
            # Engine-local copies of the resample weights: keeps the
            # wconst-DMA semaphore off the stage TT ops (gen3 TT has only
            # 2 sync-wait slots, and each load tile already needs 2).
            wc_v = cpool.tile([128, O_C1], BF16)
            wc_g = cpool.tile([128, 64], BF16)
            nc.vector.tensor_copy(out=wc_v[:], in_=wc[:, 0:O_C1])
            nc.gpsimd.tensor_copy(out=wc_g[:], in_=wc[:, O_W1B:O_W1B + 64])

            r_par = ppool.tile([128, NSLOT, XPAD], BF16)
            h_par = ppool.tile([128, NSLOT, XPAD], BF16)
            for tile in (r_par, h_par):
                nc.vector.memset(tile[0:64, 64, :], 0.0)    # E slot 64 pad
                nc.vector.memset(tile[64:128, 0, :], 0.0)   # O slot 0 pad
                nc.vector.memset(tile[:, :, 0], 0.0)        # left col pad
                nc.vector.memset(tile[:, :, 129], 0.0)      # right col pad

            c1_done = c2_done = 0

            def conv_progress(c1_max):
                nonlocal c1_done, c2_done
                while c1_done < c1_max:
                    _emit_conv1_pair(nc, wc, r_par, h_par, psum_pool, c1_done)
                    c1_done += 1
                while c2_done < c1_done - 1:
                    _emit_conv2_pair(nc, wc, h_par, out_view, out_pool,
                                     psum_pool, c2_done)
                    c2_done += 1

            mul = mybir.AluOpType.mult
            add = mybir.AluOpType.add
            for kc in range(NCHUNK):
                s0 = SCHUNK * kc
                fa = ldpool.tile([128, SCHUNK, 256], F32, tag="fa", bufs=2)
                fb = ldpool.tile([128, SCHUNK, 256], F32, tag="fb", bufs=2)
                # partition p = (par, c); row = 4s + 2par (+1 for fb)
                nc.sync.dma_start(out=fa[0:64], in_=feat_v[0, :, bass.ds(s0, SCHUNK), :])
                nc.sync.dma_start(out=fa[64:128], in_=feat_v[2, :, bass.ds(s0, SCHUNK), :])
                nc.sync.dma_start(out=fb[0:64], in_=feat_v[1, :, bass.ds(s0, SCHUNK), :])
                nc.sync.dma_start(out=fb[64:128], in_=feat_v[3, :, bass.ds(s0, SCHUNK), :])

                wa = wc_v[:, bass.ds(O_W1A + s0, SCHUNK)].unsqueeze(2).broadcast_to(
                    [128, SCHUNK, 256])
                wb = wc_g[:, bass.ds(s0, SCHUNK)].unsqueeze(2).broadcast_to(
                    [128, SCHUNK, 256])
                wi = wc_v[:, bass.ds(O_WINT, 256)].unsqueeze(1).broadcast_to(
                    [128, SCHUNK, 256])

                t1 = stpool.tile([128, SCHUNK, 256], BF16, tag="t1", bufs=2)
                t2 = stpool.tile([128, SCHUNK, 256], BF16, tag="t2", bufs=2)
                t3 = stpool.tile([128, SCHUNK, 256], BF16, tag="t3", bufs=2)
                tp = stpool.tile([128, SCHUNK, 256], BF16, tag="tp", bufs=2)
                nc.vector.tensor_tensor(out=t1[:], in0=fa[:], in1=wa, op=mul)
                nc.gpsimd.tensor_tensor(out=t2[:], in0=fb[:], in1=wb, op=mul)
                nc.vector.tensor_tensor(out=t3[:], in0=t1[:], in1=t2[:], op=add)
                nc.vector.tensor_tensor(out=tp[:], in0=t3[:], in1=wi, op=mul)
                # pairwise sum of adjacent cols -> r_par (E half / O half +1)
                tpe = tp[:, :, 0::2]
                tpo = tp[:, :, 1::2]
                nc.vector.tensor_tensor(
                    out=r_par[0:64, bass.ds(s0, SCHUNK), 1:129],
                    in0=tpe[0:64], in1=tpo[0:64], op=add)
                nc.vector.tensor_tensor(
                    out=r_par[64:128, bass.ds(s0 + 1, SCHUNK), 1:129],
                    in0=tpe[64:128], in1=tpo[64:128], op=add)

                if kc < NCHUNK - 1:
                    conv_progress(min(2 * kc + 1, 16))
            conv_progress(16)
            # final conv2 pair: its E-slot-64 halo is the memset pad,
            # not a 17th conv1 pair, so emit it explicitly
            while c2_done < 16:
                _emit_conv2_pair(nc, wc, h_par, out_view, out_pool,
                                 psum_pool, c2_done)
                c2_done += 1

    nc.finalize()
    return nc


_PROG = None


def _get_program():
    global _PROG
    if _PROG is None:
        _PROG = build_program()
    return _PROG


def _bias_map(conv1_b, conv2_b, conv2_w):
    """Feat-independent bias contribution of both convs, [3,128,128]."""
    if not conv1_b.any() and not conv2_b.any():
        return None
    h = np.broadcast_to(conv1_b[:, None, None], (C, HO, HO)).astype(np.float32)
    hp = np.zeros((C, HO + 2, HO + 2), np.float32)
    hp[:, 1:-1, 1:-1] = h
    o = np.zeros((3, HO, HO), np.float32)
    for kh in range(3):
        for kw in range(3):
            o += np.einsum("oc,chw->ohw", conv2_w[:, :, kh, kw],
                           hp[:, kh:kh + HO, kw:kw + HO])
    return o + conv2_b[:, None, None]


def kernel(**inputs):
    feat = np.ascontiguousarray(np.asarray(inputs["feat"], dtype=np.float32))
    conv1_w = np.asarray(inputs["conv1_w"], np.float32)
    conv1_b = np.asarray(inputs["conv1_b"], np.float32)
    conv2_w = np.asarray(inputs["conv2_w"], np.float32)
    conv2_b = np.asarray(inputs["conv2_b"], np.float32)

    wc = _build_wconst(conv1_w, conv2_w)
    nc = _get_program()
    in_maps = [{"feat": feat[b], "wconst": wc} for b in range(feat.shape[0])]
    import os
    trace = bool(int(os.environ.get("AFR_TRACE", "0")))
    res = run_bass_kernel_spmd(nc, in_maps, core_ids=list(range(8)),
                               trace=trace)
    if trace:
        print(f"HW exec time: {res.exec_time_ns} ns")
    outs = np.stack([m["out"].reshape(3, HO, HO) for m in res.results])
    bm = _bias_map(conv1_b, conv2_b, conv2_w)
    if bm is not None:
        outs = outs + bm[None]
    return outs.astype(np.float32)


if __name__ == "__main__":
    prog = build_program()
    print("program built OK")


# revision 23
# speedup vs baseline: 1.6667x; 1.0164x over previous
"""AFResampler Trainium2 kernel.

Math: the reference's _normalize() is shift-invariant, so all 9 (oh, ow)
offsets produce the SAME sampling grid; the MLP-weighted sum then cancels
exactly (value / w_sum == single grid_sample).  With H=W=256 -> 128, the
grid sample reduces to a separable 2x bilinear downsample:

    r[i]    = (1 - i/127) * feat[2i]   + (i/127) * feat[2i+1]      (rows)
    r[:, j] = (1 - j/127) * rr[:, 2j]  + (j/127) * rr[:, 2j+1]     (cols)

followed by out = conv3x3(conv3x3(r, w1)+b1, w2)+b2.  Bias contributions
are feat-independent and added on the host.

Device layout: one batch element per NeuronCore (8-way data parallel).
On-chip, partitions = (row-parity, channel): p<64 holds channel c's EVEN
r-rows, p>=64 the ODD r-rows (O-array slot s = r[2s-1], so conv taps for
one output row always read a single slot index across both halves).
3x3 convs run as 2 matmuls per kernel-column (one K=128, one K=64)
per 4-row output tile, accumulated in PSUM, bf16 operands / f32 psum.
"""

import numpy as np

import concourse.bass as bass
import concourse.bacc as bacc
import concourse.mybir as mybir
from concourse.tile import TileContext
from concourse.bass_utils import run_bass_kernel_spmd

BF16 = mybir.dt.bfloat16
F32 = mybir.dt.float32
NP_BF16 = np.dtype(mybir.dt.np(BF16))

C = 64          # channels
HO = 128        # output spatial
NSLOT = 65      # parity slots incl pad
XPAD = 130      # 128 + 2 zero cols
SCHUNK = 8      # slots computed per resample chunk
NCHUNK = 64 // SCHUNK

# wconst free-dim offsets (one [128, F] tensor holds all weights)
O_W1A = 0            # [128, 64]  stage1 A weights
O_W1B = 64           # [128, 64]
O_WINT = 128         # [128, 256] interleaved stage2 weights
O_C1 = 384           # 12 slabs x 128
C2W = 68             # conv2 slab width (M cols 0..2 even / 64..66 odd)
O_C2 = O_C1 + 12 * 128   # 12 slabs x C2W
WF = O_C2 + 12 * C2W     # total free size


def _build_wconst(conv1_w, conv2_w):
    """Assemble the [128, WF] bf16 constant tensor."""
    wc = np.zeros((128, WF), np.float32)
    s = np.arange(64, dtype=np.float32)
    par = (np.arange(128) // 64).astype(np.float32)[:, None]  # [128,1]
    yw = (2.0 * s[None, :] + par) / 127.0                     # B weight per (p, slot)
    wc[:, O_W1A:O_W1A + 64] = 1.0 - yw
    wc[:, O_W1B:O_W1B + 64] = yw
    j = np.arange(128, dtype=np.float32) / 127.0
    wint = np.zeros(256, np.float32)
    wint[0::2] = 1.0 - j
    wint[1::2] = j
    wc[:, O_WINT:O_WINT + 256] = wint[None, :]

    def t(w, kh, kw):  # lhsT block [cin, cout]
        return w[:, :, kh, kw].T.astype(np.float32)

    # conv1 slabs (12 x [128, 128]).  M-cols 0..63 for even tiles,
    # 64..127 for odd tiles (psum partition offset trick).
    for dw in range(3):
        ea = np.zeros((128, 128), np.float32)
        ea[0:64, 0:64] = t(conv1_w, 1, dw)     # E half: tap dh=0
        ea[64:128, 0:64] = t(conv1_w, 0, dw)   # O half: tap dh=-1
        wc[:, O_C1 + dw * 128: O_C1 + dw * 128 + 128] = ea
        eb = np.zeros((128, 128), np.float32)
        eb[64:128, 0:64] = t(conv1_w, 2, dw)   # O slot s+1: tap dh=+1
        wc[:, O_C1 + (3 + dw) * 128: O_C1 + (4 + dw) * 128] = eb
        oa = np.zeros((128, 128), np.float32)
        oa[0:64, 64:128] = t(conv1_w, 2, dw)   # E slot s+1: tap dh=+1
        oa[64:128, 64:128] = t(conv1_w, 1, dw) # O slot s+1: tap dh=0
        wc[:, O_C1 + (6 + dw) * 128: O_C1 + (7 + dw) * 128] = oa
        ob = np.zeros((128, 128), np.float32)
        ob[0:64, 64:128] = t(conv1_w, 0, dw)   # E slot s: tap dh=-1
        wc[:, O_C1 + (9 + dw) * 128: O_C1 + (10 + dw) * 128] = ob

    # conv2 slabs (12 x [128, 68]).  Even tiles use M-cols 0..2, odd tiles
    # 64..66 so even/odd matmuls col-pack into disjoint array col-groups.
    for dw in range(3):
        ea = np.zeros((128, C2W), np.float32)
        ea[0:64, 0:3] = t(conv2_w, 1, dw)
        ea[64:128, 0:3] = t(conv2_w, 0, dw)
        wc[:, O_C2 + dw * C2W: O_C2 + (dw + 1) * C2W] = ea
        eb = np.zeros((128, C2W), np.float32)
        eb[64:128, 0:3] = t(conv2_w, 2, dw)
        wc[:, O_C2 + (3 + dw) * C2W: O_C2 + (4 + dw) * C2W] = eb
        oa = np.zeros((128, C2W), np.float32)
        oa[0:64, 64:67] = t(conv2_w, 2, dw)
        oa[64:128, 64:67] = t(conv2_w, 1, dw)
        wc[:, O_C2 + (6 + dw) * C2W: O_C2 + (7 + dw) * C2W] = oa
        ob = np.zeros((128, C2W), np.float32)
        ob[0:64, 64:67] = t(conv2_w, 0, dw)
        wc[:, O_C2 + (9 + dw) * C2W: O_C2 + (10 + dw) * C2W] = ob
    return wc.astype(NP_BF16)


def _emit_conv1_pair(nc, wc, r_par, h_par, psum_pool, t):
    """Even+odd conv1 tiles for slot-group t, col-packed into one psum bank
    (even rows -> psum partitions 0..63 / array cols 0..63; odd rows ->
    64..127).  Matmuls of the two halves interleave so the PE runs them
    concurrently in disjoint column groups."""
    ps = psum_pool.tile([128, 4, 128], F32, tag="p1", bufs=3,
                        name=f"ps1_{t}")
    for dw in range(3):
        # even tile: taps dh=0 (E slot s) + dh=-1 (O slot s), K=128
        nc.tensor.matmul(ps[0:64],
                         wc[:, bass.ds(O_C1 + dw * 128, 64)],
                         r_par[:, bass.ds(4 * t, 4), bass.ds(dw, 128)],
                         start=(dw == 0), stop=False)
        # odd tile: taps dh=+1 (E slot s+1) + dh=0 (O slot s+1), K=128
        nc.tensor.matmul(ps[64:128],
                         wc[:, bass.ds(O_C1 + (6 + dw) * 128 + 64, 64)],
                         r_par[:, bass.ds(4 * t + 1, 4), bass.ds(dw, 128)],
                         start=(dw == 0), stop=False)
        # even tile: tap dh=+1 (O slot s+1), K=64
        nc.tensor.matmul(ps[0:64],
                         wc[64:128, bass.ds(O_C1 + (3 + dw) * 128, 64)],
                         r_par[64:128, bass.ds(4 * t + 1, 4), bass.ds(dw, 128)],
                         start=False, stop=(dw == 2))
        # odd tile: tap dh=-1 (E slot s), K=64
        nc.tensor.matmul(ps[64:128],
                         wc[0:64, bass.ds(O_C1 + (9 + dw) * 128 + 64, 64)],
                         r_par[0:64, bass.ds(4 * t, 4), bass.ds(dw, 128)],
                         start=False, stop=(dw == 2))
    nc.scalar.activation(
        h_par[0:64, bass.ds(4 * t, 4), 1:129], ps[0:64, :, :],
        mybir.ActivationFunctionType.Copy)
    nc.scalar.activation(
        h_par[64:128, bass.ds(4 * t + 1, 4), 1:129], ps[64:128, :, :],
        mybir.ActivationFunctionType.Copy)


def _emit_conv2_pair(nc, wc, h_par, out_sb, psum_pool, t):
    ps = psum_pool.tile([128, 4, 128], F32, tag="p2", bufs=3,
                        name=f"ps2_{t}")
    for dw in range(3):
        nc.tensor.matmul(ps[0:3],
                         wc[:, bass.ds(O_C2 + dw * C2W, 3)],
                         h_par[:, bass.ds(4 * t, 4), bass.ds(dw, 128)],
                         start=(dw == 0), stop=False)
        nc.tensor.matmul(ps[64:67],
                         wc[:, bass.ds(O_C2 + (6 + dw) * C2W + 64, 3)],
                         h_par[:, bass.ds(4 * t + 1, 4), bass.ds(dw, 128)],
                         start=(dw == 0), stop=False)
        nc.tensor.matmul(ps[0:3],
                         wc[64:128, bass.ds(O_C2 + (3 + dw) * C2W, 3)],
                         h_par[64:128, bass.ds(4 * t + 1, 4), bass.ds(dw, 128)],
                         start=False, stop=(dw == 2))
        nc.tensor.matmul(ps[64:67],
                         wc[0:64, bass.ds(O_C2 + (9 + dw) * C2W + 64, 3)],
                         h_par[0:64, bass.ds(4 * t, 4), bass.ds(dw, 128)],
                         start=False, stop=(dw == 2))
    nc.scalar.activation(out_sb[0:3, bass.ds(4 * t, 4), :], ps[0:3, :, :],
                         mybir.ActivationFunctionType.Copy)
    nc.scalar.activation(out_sb[64:67, bass.ds(4 * t, 4), :], ps[64:67, :, :],
                         mybir.ActivationFunctionType.Copy)


def build_program():
    nc = bacc.Bacc(trn_type="TRN2")
    feat = nc.dram_tensor("feat", [C, 256, 256], F32, kind="ExternalInput")
    wconst = nc.dram_tensor("wconst", [128, WF], BF16, kind="ExternalInput")
    out = nc.dram_tensor("out", [3, HO, HO], F32, kind="ExternalOutput")
    # feat viewed as [q, c, s4, x] with y = 4*s4 + q
    feat_v = feat[:].rearrange("c (s q) x -> q c s x", q=4)
    out_view = out[:].rearrange("co (s two) x -> co s two x", two=2)

    with TileContext(nc) as tc:
        with (
            tc.tile_pool(name="const", bufs=1) as cpool,
            tc.tile_pool(name="persist", bufs=1) as ppool,
            tc.tile_pool(name="ld", bufs=2) as ldpool,
            tc.tile_pool(name="st1", bufs=2) as stpool,
            tc.tile_pool(name="psum", bufs=2, space="PSUM") as psum_pool,
        ):
            wc = cpool.tile([128, WF], BF16)
            nc.sync.dma_start(out=wc[:], in_=wconst[:])
            # Engine-local copies of the resample weights: keeps the
            # wconst-DMA semaphore off the stage TT ops (gen3 TT has only
            # 2 sync-wait slots, and each load tile already needs 2).
            wc_v = cpool.tile([128, O_C1], BF16)
            wc_g = cpool.tile([128, 64], BF16)
            nc.vector.tensor_copy(out=wc_v[:], in_=wc[:, 0:O_C1])
            nc.gpsimd.tensor_copy(out=wc_g[:], in_=wc[:, O_W1B:O_W1B + 64])

            r_par = ppool.tile([128, NSLOT, XPAD], BF16)
            h_par = ppool.tile([128, NSLOT, XPAD], BF16)
            out_sb = ppool.tile([67, 64, 128], F32)
            for tile in (r_par, h_par):
                nc.vector.memset(tile[0:64, 64, :], 0.0)    # E slot 64 pad
                nc.vector.memset(tile[64:128, 0, :], 0.0)   # O slot 0 pad
                nc.vector.memset(tile[:, :, 0], 0.0)        # left col pad
                nc.vector.memset(tile[:, :, 129], 0.0)      # right col pad

            c1_done = c2_done = 0

            def conv_progress(c1_max):
                nonlocal c1_done, c2_done
                while c1_done < c1_max:
                    _emit_conv1_pair(nc, wc, r_par, h_par, psum_pool, c1_done)
                    c1_done += 1
                # lag conv2 so h_par evacuations complete well before the
                # conv2 matmuls need them (keeps the PE stream stall-free)
                while c2_done < c1_done - 3:
                    _emit_conv2_pair(nc, wc, h_par, out_sb, psum_pool, c2_done)
                    c2_done += 1

            mul = mybir.AluOpType.mult
            add = mybir.AluOpType.add
            for kc in range(NCHUNK):
                s0 = SCHUNK * kc
                fa = ldpool.tile([128, SCHUNK, 256], F32, tag="fa", bufs=2)
                fb = ldpool.tile([128, SCHUNK, 256], F32, tag="fb", bufs=2)
                # partition p = (par, c); row = 4s + 2par (+1 for fb)
                nc.sync.dma_start(out=fa[0:64], in_=feat_v[0, :, bass.ds(s0, SCHUNK), :])
                nc.sync.dma_start(out=fa[64:128], in_=feat_v[2, :, bass.ds(s0, SCHUNK), :])
                nc.scalar.dma_start(out=fb[0:64], in_=feat_v[1, :, bass.ds(s0, SCHUNK), :])
                nc.scalar.dma_start(out=fb[64:128], in_=feat_v[3, :, bass.ds(s0, SCHUNK), :])

                wa = wc_v[:, bass.ds(O_W1A + s0, SCHUNK)].unsqueeze(2).broadcast_to(
                    [128, SCHUNK, 256])
                wb = wc_g[:, bass.ds(s0, SCHUNK)].unsqueeze(2).broadcast_to(
                    [128, SCHUNK, 256])
                wi = wc_v[:, bass.ds(O_WINT, 256)].unsqueeze(1).broadcast_to(
                    [128, SCHUNK, 256])

                t1 = stpool.tile([128, SCHUNK, 256], BF16, tag="t1", bufs=2)
                t2 = stpool.tile([128, SCHUNK, 256], BF16, tag="t2", bufs=2)
                t3 = stpool.tile([128, SCHUNK, 256], BF16, tag="t3", bufs=2)
                tp = stpool.tile([128, SCHUNK, 256], BF16, tag="tp", bufs=2)
                nc.vector.tensor_tensor(out=t1[:], in0=fa[:], in1=wa, op=mul)
                nc.gpsimd.tensor_tensor(out=t2[:], in0=fb[:], in1=wb, op=mul)
                nc.vector.tensor_tensor(out=t3[:], in0=t1[:], in1=t2[:], op=add)
                nc.vector.tensor_tensor(out=tp[:], in0=t3[:], in1=wi, op=mul)
                # pairwise sum of adjacent cols -> r_par (E half / O half +1)
                tpe = tp[:, :, 0::2]
                tpo = tp[:, :, 1::2]
                nc.vector.tensor_tensor(
                    out=r_par[0:64, bass.ds(s0, SCHUNK), 1:129],
                    in0=tpe[0:64], in1=tpo[0:64], op=add)
                nc.gpsimd.tensor_tensor(
                    out=r_par[64:128, bass.ds(s0 + 1, SCHUNK), 1:129],
                    in0=tpe[64:128], in1=tpo[64:128], op=add)

                if kc < NCHUNK - 1:
                    conv_progress(min(2 * kc + 1, 16))
            conv_progress(16)
            # remaining conv2 pairs (the final one's E-slot-64 halo is the
            # memset pad, not a 17th conv1 pair)
            while c2_done < 16:
                _emit_conv2_pair(nc, wc, h_par, out_sb, psum_pool, c2_done)
                c2_done += 1
            nc.sync.dma_start(out=out_view[:, :, 0, :], in_=out_sb[0:3])
            nc.sync.dma_start(out=out_view[:, :, 1, :], in_=out_sb[64:67])

    nc.finalize()
    return nc


_PROG = None


def _get_program():
    global _PROG
    if _PROG is None:
        _PROG = build_program()
    return _PROG


def _bias_map(conv1_b, conv2_b, conv2_w):
    """Feat-independent bias contribution of both convs, [3,128,128]."""
    if not conv1_b.any() and not conv2_b.any():
        return None
    h = np.broadcast_to(conv1_b[:, None, None], (C, HO, HO)).astype(np.float32)
    hp = np.zeros((C, HO + 2, HO + 2), np.float32)
    hp[:, 1:-1, 1:-1] = h
    o = np.zeros((3, HO, HO), np.float32)
    for kh in range(3):
        for kw in range(3):
            o += np.einsum("oc,chw->ohw", conv2_w[:, :, kh, kw],
                           hp[:, kh:kh + HO, kw:kw + HO])
    return o + conv2_b[:, None, None]


def kernel(**inputs):
    feat = np.ascontiguousarray(np.asarray(inputs["feat"], dtype=np.float32))
    conv1_w = np.asarray(inputs["conv1_w"], np.float32)
    conv1_b = np.asarray(inputs["conv1_b"], np.float32)
    conv2_w = np.asarray(inputs["conv2_w"], np.float32)
    conv2_b = np.asarray(inputs["conv2_b"], np.float32)

    wc = _build_wconst(conv1_w, conv2_w)
    nc = _get_program()
    in_maps = [{"feat": feat[b], "wconst": wc} for b in range(feat.shape[0])]
    import os
    trace = bool(int(os.environ.get("AFR_TRACE", "0")))
    res = run_bass_kernel_spmd(nc, in_maps, core_ids=list(range(8)),
                               trace=trace)
    if trace:
        print(f"HW exec time: {res.exec_time_ns} ns")
    outs = np.stack([m["out"].reshape(3, HO, HO) for m in res.results])
    bm = _bias_map(conv1_b, conv2_b, conv2_w)
    if bm is not None:
        outs = outs + bm[None]
    return outs.astype(np.float32)


if __name__ == "__main__":
    prog = build_program()
    print("program built OK")
